# revision 1
# baseline (speedup 1.0000x reference)
"""Embedding lookup (gather) on 8 Trainium2 NeuronCores.

Strategy: data-parallel. The [768, 50257] table is transposed host-side to
row-major [50257, 768] and replicated to every core's DRAM; the 8*2048 = 16384
token indices are sharded 2048 per core. Each core gathers its 2048 embedding
rows from its local table copy with indirect DMA (SWDGE) into SBUF, then
streams them out to its output shard with HWDGE stores. No collectives needed.

Raw Bass (no TileContext, no nc.Block): all-engine barriers cost ~3-4 us each
on a ~40 us kernel, so the init barrier + const memsets are stripped from the
module and engine streams are left unsynchronized except for the DMA
semaphores that express real data dependencies:
  - SP loads the indices in three slices (column 0 first, so Q7 can start
    generating gather 0's descriptors ASAP; one sem per DMA), then stores
    each gathered group, alternating with ACT's HWDGE ring (ssem counts all).
  - Pool/GpSimd (SWDGE) waits for the indices, then issues the 16 indirect
    gathers back-to-back, round-robin over 4 SWDGE queues so each SDMA
    engine keeps several gather packets in flight (hides random-row HBM
    latency). All 16 groups are fully buffered in SBUF (48 KB/partition),
    so gathers never wait on stores.
  - Store i waits its gather's dedicated sem (gsems[i] >= 16). Cumulative
    counts across SWDGE DMAs on one sem are unsound: the 16 increments per
    DMA come from 16 independently-progressing SDMA engines.
  - SP's final cumulative wait on ssem (sound: it is the maximum total)
    covers all stores on both rings before the program retires.

Per-core HBM traffic: ~6.3 MB gather read + ~6.3 MB store write -> the kernel
is DMA/HBM-roofline bound (~44 us: ~6 us NEFF launch, ~22 us serial Q7
descriptor generation feeding ~33 us/engine of DMA work, ~2 us store tail).
"""

import numpy as np

VOCAB = 50257
EMBED = 768
BATCH = 8
SEQ = 2048
N_CORES = 8
P = 128                      # SBUF partitions
TOK_PER_CORE = BATCH * SEQ // N_CORES   # 2048
GROUPS = TOK_PER_CORE // P              # 16 gather groups of 128 rows

_cached = {}
LAST_RESULTS = None  # BassKernelResults of the most recent run (for test harness)


def _build():
    """Build + compile the single-core Bass program (shared SPMD across 8 cores)."""
    import concourse.bacc as bacc
    import concourse.bass as bass
    from concourse import mybir

    nc = bacc.Bacc(
        "TRN2",
        target_bir_lowering=False,
        debug=False,
        num_devices=N_CORES,
        num_swdge_queues=4,
    )

    # Drop the init-time const memsets and the all-engine barrier (~3.5 us):
    # nothing in this kernel reads the const APs, and the engine streams only
    # communicate through DMA semaphores which the loader zero-initializes.
    main_blk = nc.m.functions[0].blocks[0]
    removable = [
        inst
        for inst in main_blk.instructions
        if type(inst).__name__ in ("InstMemset", "InstDrain", "InstEventSemaphore")
    ]
    for inst in removable:
        main_blk.instructions.remove(inst)

    table = nc.dram_tensor(
        "table", [VOCAB, EMBED], mybir.dt.float32, kind="ExternalInput"
    ).ap()
    idx = nc.dram_tensor(
        "idx", [P, GROUPS], mybir.dt.int32, kind="ExternalInput"
    ).ap()
    out = nc.dram_tensor(
        "out", [GROUPS, P, EMBED], mybir.dt.float32, kind="ExternalOutput"
    ).ap()

    import contextlib

    with contextlib.ExitStack() as ctx:
        idx_sb = ctx.enter_context(
            nc.sbuf_tensor("idx_sb", [P, GROUPS], mybir.dt.int32)
        )
        emb = ctx.enter_context(
            nc.sbuf_tensor("emb", [P, GROUPS * EMBED], mybir.dt.float32)
        )
        isem = ctx.enter_context(nc.semaphore("isem"))
        isem2 = ctx.enter_context(nc.semaphore("isem2"))
        isem3 = ctx.enter_context(nc.semaphore("isem3"))
        ssem = ctx.enter_context(nc.semaphore("ssem"))
        # One completion sem PER gather: a single SWDGE DMA's 16 increments
        # come from 16 independently-progressing SDMA engines, so cumulative
        # counts across DMAs on one sem do NOT imply per-DMA completion
        # (engine A can contribute several increments while engine B still
        # drains an earlier DMA). Same convention Tile uses (DMASWx lanes).
        # NOTE: the HW indirect DMA honors only the offset AP's partition dim
        # (<=128 indices per instruction) - a [128, 2] offset AP silently
        # drops the second column - so gathers are fixed at 128 rows each.
        gsems = [
            ctx.enter_context(nc.semaphore(f"gsem{i}")) for i in range(GROUPS)
        ]

        # SP: index load first (HWDGE - cheap descriptor gen, Q7 stays free).
        # Column 0 ships alone so Q7 can start generating gather 0's
        # descriptors at the earliest possible moment; the rest follows in
        # two slices that land during the first generations. One sem per DMA.
        H = GROUPS // 2
        with nc.allow_non_contiguous_dma(
            reason="column 0 of the idx matrix: 128 x 4B, latency-bound either way"
        ):
            nc.sync.dma_start(idx_sb[:, :1], idx[:, :1]).then_inc(isem, 16)
        nc.sync.dma_start(idx_sb[:, 1:H], idx[:, 1:H]).then_inc(isem2, 16)
        nc.sync.dma_start(idx_sb[:, H:], idx[:, H:]).then_inc(isem3, 16)

        # Pool/SWDGE: 16 indirect gathers, fully buffered, no store waits.
        nc.gpsimd.wait_ge(isem, 16)
        for i in range(GROUPS):
            if i == 1:
                nc.gpsimd.wait_ge(isem2, 16)
            if i == H:
                nc.gpsimd.wait_ge(isem3, 16)
            gi = nc.gpsimd.indirect_dma_start(
                out=emb[:, i * EMBED : (i + 1) * EMBED],
                out_offset=None,
                in_=table[:],
                in_offset=bass.IndirectOffsetOnAxis(ap=idx_sb[:, i : i + 1], axis=0),
            )
            # Alternate the two SWDGE rings so each SDMA engine holds gather
            # packets from both and round-robins between them - more
            # outstanding HBM reads per engine hides random-row latency.
            if i % 4:
                gi.ins.queue = f"qPoolDynamic{i % 4}"
            gi.then_inc(gsems[i], 16)

        # Stores: alternate the two HWDGE rings (SP=qSPDynamicHW,
        # ACT=qActDynamicHW) so more store packets are in flight per SDMA
        # engine while gather packets round-robin on the SWDGE ring.
        for i in range(GROUPS):
            eng = nc.sync if i % 2 == 0 else nc.scalar
            eng.wait_ge(gsems[i], 16)
            eng.dma_start(out[i], emb[:, i * EMBED : (i + 1) * EMBED]).then_inc(
                ssem, 16
            )

        # All stores landed (sem increments fire after last-byte receipt).
        # A cumulative wait is sound here: GROUPS*16 is the maximum total.
        nc.sync.wait_ge(ssem, GROUPS * 16)

    nc.compile()
    return nc


def _ensure_axon_hooks_importable():
    """bass_utils imports antenv.axon_hooks when BASS_TRACE is set under axon;
    the agent image's antenv package lacks that module. Provide a no-op shim
    so a stray BASS_TRACE env var cannot crash the run (tracing degrades)."""
    import sys
    import types

    try:
        import antenv.axon_hooks  # noqa: F401
        return
    except ImportError:
        pass
    try:
        import antenv
    except ImportError:
        return
    mod = types.ModuleType("antenv.axon_hooks")
    _h = [None]
    mod.set_axon_ntff_profile_hook = lambda h: _h.__setitem__(0, h)
    mod.get_axon_ntff_profile_hook = lambda: _h[0]
    sys.modules["antenv.axon_hooks"] = mod
    antenv.axon_hooks = mod


def kernel(x, weight):
    global LAST_RESULTS
    _ensure_axon_hooks_importable()
    from concourse.bass_utils import run_bass_kernel_spmd

    if "nc" not in _cached:
        _cached["nc"] = _build()
    nc = _cached["nc"]

    # Host-side input staging: transpose table to row-major [V, D]; shard
    # tokens 2048/core, laid out [128 partitions, 16 groups] so group g of
    # core c covers tokens c*2048 + g*128 + p.
    wt = np.ascontiguousarray(np.asarray(weight, dtype=np.float32).T)
    x_flat = np.asarray(x, dtype=np.int32).reshape(N_CORES, TOK_PER_CORE)
    in_maps = []
    for c in range(N_CORES):
        idx_c = np.ascontiguousarray(x_flat[c].reshape(GROUPS, P).T)
        in_maps.append({"table": wt, "idx": idx_c})

    res = run_bass_kernel_spmd(nc, in_maps, core_ids=list(range(N_CORES)))
    LAST_RESULTS = res

    out = np.empty((N_CORES, TOK_PER_CORE, EMBED), dtype=np.float32)
    for c in range(N_CORES):
        out[c] = np.asarray(res.results[c]["out"]).reshape(TOK_PER_CORE, EMBED)
    return out.reshape(BATCH, SEQ, EMBED)



# revision 3
# speedup vs baseline: 1.2387x; 1.2387x over previous
"""Embedding lookup (gather) on 8 Trainium2 NeuronCores.

Strategy: data-parallel, bf16 table, int32 indirect gather, DVE upconvert.

The [768, 50257] f32 table is transposed and converted to bf16 host-side
(the tolerance is rel_err < 2e-2; bf16 rounds at 2^-9 ~ 0.2% and, unlike
fp16, has no subnormal blow-up for near-zero weights) and replicated to
every core's DRAM as row-major [50257, 768] bf16. The 8*2048 = 16384 token
indices are sharded 2048 per core, 16 gather groups of 128 rows.

Rate analysis (measured on this part): SWDGE descriptor generation is
engine-serial on GpSimd at ~1.4us per 128-row indirect_dma_start, and a
gather's DMAs only fire once its own descgen finishes -> the 16 gathers
issue over ~22.6us no matter what, and that chain is the kernel's spine.
With an f32 table (the previous kernel) the issue pace put 279 GB/s of
gather + 279 GB/s of store demand on a ~420 GB/s DMA fabric, so everything
slipped and the kernel ran 47.7us. With bf16 the gather stream halves to
~139 GB/s, stores ride along at ~279 GB/s, the fabric is exactly fed, and
the kernel tracks the descgen spine + a short tail.

Per group: SWDGE gather (bf16, 196KB) -> DVE tensor_copy upconvert to f32
(~0.4us, otherwise-idle engine) -> HWDGE store (f32, 393KB) alternating the
SP/ACT rings. Per-core fabric traffic: 3.15 MB gather + 6.3 MB store.

Raw Bass (no TileContext, no nc.Block): all-engine barriers cost ~3-4 us
each on a ~35 us kernel, so the init barrier + const memsets are stripped
from the module and engine streams are left unsynchronized except for the
semaphores that express real data dependencies:
  - SP loads the indices in three slices (column 0 first, so Q7 can start
    generating gather 0's descriptors ASAP; one sem per DMA).
  - GpSimd waits for the indices, then issues the 16 indirect gathers
    back-to-back (descgen-paced), round-robin over 4 SWDGE queues.
  - Gather i completes on its dedicated sem (gsems[i] >= 16; cumulative
    counts across SWDGE DMAs on one sem are unsound - the 16 increments
    per DMA come from 16 independently-progressing SDMA engines).
  - DVE converts group i after gsems[i]; its EVSEM (vsem) counts in group
    order from the single DVE stream, so cumulative store waits are sound.
  - SP's final cumulative wait on ssem covers all stores before retire.
NOTE: the HW indirect DMA honors only the offset AP's partition dim
(<=128 indices per instruction) - a [128, 2] offset AP silently drops the
second column - so gathers are fixed at 128 rows each.
"""

import numpy as np

VOCAB = 50257
EMBED = 768
BATCH = 8
SEQ = 2048
N_CORES = 8
P = 128                      # SBUF partitions
TOK_PER_CORE = BATCH * SEQ // N_CORES   # 2048
GROUPS = TOK_PER_CORE // P              # 16 gather groups of 128 rows

_cached = {}
LAST_RESULTS = None  # BassKernelResults of the most recent run (for test harness)


def _build():
    """Build + compile the single-core Bass program (shared SPMD across 8 cores)."""
    import concourse.bacc as bacc
    import concourse.bass as bass
    from concourse import mybir

    nc = bacc.Bacc(
        "TRN2",
        target_bir_lowering=False,
        debug=False,
        num_devices=N_CORES,
        num_swdge_queues=4,
    )

    # Drop the init-time const memsets and the all-engine barrier (~3.5 us):
    # nothing in this kernel reads the const APs, and the engine streams only
    # communicate through DMA semaphores which the loader zero-initializes.
    main_blk = nc.m.functions[0].blocks[0]
    removable = [
        inst
        for inst in main_blk.instructions
        if type(inst).__name__ in ("InstMemset", "InstDrain", "InstEventSemaphore")
    ]
    for inst in removable:
        main_blk.instructions.remove(inst)

    table = nc.dram_tensor(
        "table", [VOCAB, EMBED], mybir.dt.bfloat16, kind="ExternalInput"
    ).ap()
    idx = nc.dram_tensor(
        "idx", [P, GROUPS], mybir.dt.int32, kind="ExternalInput"
    ).ap()
    out = nc.dram_tensor(
        "out", [GROUPS, P, EMBED], mybir.dt.float32, kind="ExternalOutput"
    ).ap()

    import contextlib

    with contextlib.ExitStack() as ctx:
        idx_sb = ctx.enter_context(
            nc.sbuf_tensor("idx_sb", [P, GROUPS], mybir.dt.int32)
        )
        emb16 = ctx.enter_context(
            nc.sbuf_tensor("emb16", [P, GROUPS * EMBED], mybir.dt.bfloat16)
        )
        emb = ctx.enter_context(
            nc.sbuf_tensor("emb", [P, GROUPS * EMBED], mybir.dt.float32)
        )
        isem = ctx.enter_context(nc.semaphore("isem"))
        isem2 = ctx.enter_context(nc.semaphore("isem2"))
        isem3 = ctx.enter_context(nc.semaphore("isem3"))
        vsem = ctx.enter_context(nc.semaphore("vsem"))
        ssem = ctx.enter_context(nc.semaphore("ssem"))
        gsems = [
            ctx.enter_context(nc.semaphore(f"gsem{i}")) for i in range(GROUPS)
        ]

        # SP: index load first (HWDGE - cheap descriptor gen, Q7 stays free).
        # Column 0 ships alone so Q7 can start generating gather 0's
        # descriptors at the earliest possible moment; the rest follows in
        # two slices that land during the first generations.
        H = GROUPS // 2
        with nc.allow_non_contiguous_dma(
            reason="column 0 of the idx matrix: 128 x 4B, latency-bound either way"
        ):
            nc.sync.dma_start(idx_sb[:, :1], idx[:, :1]).then_inc(isem, 16)
        nc.sync.dma_start(idx_sb[:, 1:H], idx[:, 1:H]).then_inc(isem2, 16)
        nc.sync.dma_start(idx_sb[:, H:], idx[:, H:]).then_inc(isem3, 16)

        # GpSimd/SWDGE: 16 indirect gathers, back-to-back (descgen-paced).
        nc.gpsimd.wait_ge(isem, 16)
        for i in range(GROUPS):
            if i == 1:
                nc.gpsimd.wait_ge(isem2, 16)
            if i == H:
                nc.gpsimd.wait_ge(isem3, 16)
            gi = nc.gpsimd.indirect_dma_start(
                out=emb16[:, i * EMBED : (i + 1) * EMBED],
                out_offset=None,
                in_=table[:],
                in_offset=bass.IndirectOffsetOnAxis(ap=idx_sb[:, i : i + 1], axis=0),
            )
            if i % 4:
                gi.ins.queue = f"qPoolDynamic{i % 4}"
            gi.then_inc(gsems[i], 16)

        # DVE: upconvert each gathered group bf16 -> f32 (also the store
        # staging). The DVE is otherwise idle; ~0.4us per group.
        for i in range(GROUPS):
            nc.vector.wait_ge(gsems[i], 16)
            nc.vector.tensor_copy(
                emb[:, i * EMBED : (i + 1) * EMBED],
                emb16[:, i * EMBED : (i + 1) * EMBED],
            ).then_inc(vsem, 1)

        # Stores: alternate the two HWDGE rings (SP=qSyncDynamicHW,
        # ACT=qActDynamicHW). vsem counts in group order from the single DVE
        # stream, so the cumulative wait is sound.
        for i in range(GROUPS):
            eng = nc.sync if i % 2 == 0 else nc.scalar
            eng.wait_ge(vsem, i + 1)
            eng.dma_start(out[i], emb[:, i * EMBED : (i + 1) * EMBED]).then_inc(
                ssem, 16
            )

        # All stores landed (sem increments fire after last-byte receipt).
        # A cumulative wait is sound here: GROUPS*16 is the maximum total.
        nc.sync.wait_ge(ssem, GROUPS * 16)

    nc.compile()
    return nc


def _ensure_axon_hooks_importable():
    """bass_utils imports antenv.axon_hooks when BASS_TRACE is set under axon;
    the agent image's antenv package lacks that module. Provide a no-op shim
    so a stray BASS_TRACE env var cannot crash the run (tracing degrades)."""
    import sys
    import types

    try:
        import antenv.axon_hooks  # noqa: F401
        return
    except ImportError:
        pass
    try:
        import antenv
    except ImportError:
        return
    mod = types.ModuleType("antenv.axon_hooks")
    _h = [None]
    mod.set_axon_ntff_profile_hook = lambda h: _h.__setitem__(0, h)
    mod.get_axon_ntff_profile_hook = lambda: _h[0]
    sys.modules["antenv.axon_hooks"] = mod
    antenv.axon_hooks = mod


def kernel(x, weight):
    global LAST_RESULTS
    _ensure_axon_hooks_importable()
    from concourse.bass_utils import run_bass_kernel_spmd

    if "nc" not in _cached:
        _cached["nc"] = _build()
    nc = _cached["nc"]

    # Host-side input staging: transpose table to row-major [V, D] bf16;
    # shard tokens 2048/core, laid out [128 partitions, 16 groups] so group g
    # of core c covers tokens c*2048 + g*128 + p.
    import ml_dtypes

    wt = np.ascontiguousarray(
        np.asarray(weight, dtype=np.float32).T.astype(ml_dtypes.bfloat16)
    )
    x_flat = np.asarray(x, dtype=np.int32).reshape(N_CORES, TOK_PER_CORE)
    in_maps = []
    for c in range(N_CORES):
        idx_c = np.ascontiguousarray(x_flat[c].reshape(GROUPS, P).T)
        in_maps.append({"table": wt, "idx": idx_c})

    res = run_bass_kernel_spmd(nc, in_maps, core_ids=list(range(N_CORES)))
    LAST_RESULTS = res

    out = np.empty((N_CORES, TOK_PER_CORE, EMBED), dtype=np.float32)
    for c in range(N_CORES):
        out[c] = np.asarray(res.results[c]["out"]).reshape(TOK_PER_CORE, EMBED)
    return out.reshape(BATCH, SEQ, EMBED)
